# revision 27
# baseline (speedup 1.0000x reference)
"""Trainium2 Bass kernel for DecomposingAttnProcessor (pad variant).

Math (pad branch contributes exactly zero since pad tokens are zeros
projected with no bias -> k_pad = v_pad = 0):
    q = hs @ Wq.T / (temp + eps)   (scale folded into Wq on host)
    k = ehs @ Wk.T ; v = ehs @ Wv.T
    scores[c,h,s,e] = q . k        (per head, dh=64)
    w = softmax over the 4 components c (dim 0)
    o = w @ v ; out = o @ Wo.T + bo + hs

Sharding: 8 cores, split S=4096 into 512-row blocks; all 4 components of
a block stay on one core (softmax couples them). K/V computed redundantly
per core (encoder seq is only 154).

All matmuls run at N=512 (one full PSUM bank) to amortize the per-matmul
LDWEIGHTS/dispatch tax; the whole 512-row s-block is processed in one
pass (no s-halves).

Encoder layout (host-packed, 640 columns):
    cols [c*128,(c+1)*128) = component c, e in [0,128)   ("main")
    cols [512+c*32, 512+c*32+32) = component c, e in [128,154) zero-padded
    to 32 ("tail") so every matmul operand is 32-aligned on partitions.
Tail matmuls for the 4 components target disjoint PE sub-arrays
(tile_position) and run concurrently instead of serially at M=26.

The tail softmax sums over components and re-broadcasts 1/sum with two
tiny constant matmuls (lones / repl) because DVE tensor_tensor requires
both SBUF inputs to share a start partition; PSUM operands are exempt.
"""

import numpy as np
import ml_dtypes

import concourse.bass as bass
import concourse.mybir as mybir
import concourse.tile as tile
from concourse import bacc
from concourse.bass_utils import run_bass_kernel_spmd

F32 = mybir.dt.float32
BF16 = mybir.dt.bfloat16
AF = mybir.ActivationFunctionType
ALU = mybir.AluOpType

NCOMP = 4
HEADS = 24
DH = 64
D = 1536
S = 4096
E = 154
EM = 128                  # main e-rows per component
ET = E - EM               # 26 tail e-rows per component
EPS = 1e-8
NCORES = 8
SL = S // NCORES          # 512 s-rows per core (per component)
FT = D // 128             # 12 feature tiles of 128
HP = HEADS // 2           # 12 head-pairs (2 heads = 128 feature rows)
ECAT = 640                # 4*128 main + 4*32 padded tail columns
TB = 4 * EM               # 512: tail block column base


def _emit(tc):
    import os
    phases = os.environ.get("K_PHASES", "ABC")
    blevel = int(os.environ.get("K_BLEVEL", "4"))
    nc = tc.nc

    xTb = nc.declare_dram_parameter("xTb", [NCOMP, D, SL], BF16, isOutput=False)
    eT = nc.declare_dram_parameter("eT", [D, ECAT], BF16, isOutput=False)
    wqT = nc.declare_dram_parameter("wqT", [D, D], BF16, isOutput=False)
    wkT = nc.declare_dram_parameter("wkT", [D, D], BF16, isOutput=False)
    wvT = nc.declare_dram_parameter("wvT", [D, D], BF16, isOutput=False)
    woT = nc.declare_dram_parameter("woT", [D, D], BF16, isOutput=False)
    bo = nc.declare_dram_parameter("bo", [128, FT], F32, isOutput=False)
    lones = nc.declare_dram_parameter("lones", [128, 32], BF16, isOutput=False)
    repl = nc.declare_dram_parameter("repl", [32, 128], BF16, isOutput=False)
    outT = nc.declare_dram_parameter("outT", [NCOMP, D, SL], F32, isOutput=True)

    xTb_v = [xTb[c].rearrange("(f p) s -> p f s", p=128) for c in range(NCOMP)]
    eT_v = eT.rearrange("(f p) e -> p f e", p=128)
    wqT_v = wqT.rearrange("(f p) o -> p f o", p=128)
    wkT_v = wkT.rearrange("(f p) o -> p f o", p=128)
    wvT_v = wvT.rearrange("(f p) o -> p f o", p=128)
    woT_v = woT.rearrange("(f p) o -> p f o", p=128)
    outT_v = [outT[c].rearrange("(f p) s -> p f s", p=128) for c in range(NCOMP)]

    with tc.tile_pool(name="persist", bufs=1) as pp:
        # ---------------- persistent tiles ----------------
        kt_sb = [pp.tile([128, ECAT], BF16, tag="kT", bufs=FT, name=f"kt{t}")
                 for t in range(FT)]
        vm_sb = [pp.tile([128, D], BF16, tag="vm", bufs=NCOMP, name=f"vm{c}")
                 for c in range(NCOMP)]
        vt_sb = pp.tile([128, D], BF16, tag="vt", bufs=1, name="vt")
        bo_sb = pp.tile([128, FT], F32, tag="bo", bufs=1, name="bo_sb")
        nc.sync.dma_start(out=bo_sb[:], in_=bo[:])
        lones_sb = pp.tile([128, 32], BF16, tag="lones", bufs=1, name="lones_sb")
        nc.sync.dma_start(out=lones_sb[:], in_=lones[:])
        repl_sb = pp.tile([32, 128], BF16, tag="repl", bufs=1, name="repl_sb")
        nc.sync.dma_start(out=repl_sb[:], in_=repl[:])

        # x panels (Q rhs + residual source) and the first wq block live in
        # the persistent pool so their DMAs overlap phase A
        # bulk prefetches ride the scalar engine's DMA queue so they never
        # delay phase A's latency-critical eT/wk loads on the sync queue
        xh = []
        for c in range(NCOMP):
            t = pp.tile([128, FT * SL], BF16, tag="xh", bufs=NCOMP,
                        name=f"xh_{c}")
            nc.scalar.dma_start(
                out=t.rearrange("p (f s) -> p f s", f=FT), in_=xTb_v[c])
            xh.append(t)
        wq00 = pp.tile([128, FT * 128], BF16, tag="wq00", bufs=1, name="wq00")
        nc.scalar.dma_start(
            out=wq00.rearrange("p (f o) -> p f o", f=FT),
            in_=wqT_v[:, :, 0:128])

        def _phases():
            # ---------------- phase A: K^T and V ----------------
            if "A" in phases:
              with (
                tc.tile_pool(name="pha", bufs=1) as pa,
                tc.tile_pool(name="pha_psum", bufs=1, space="PSUM") as pap,
              ):
                et_b = pa.tile([128, FT * ECAT], BF16, tag="eT", bufs=1,
                               name="et_b")
                # per-fi DMAs so the first K^T chain starts early
                for fi in range(FT):
                    nc.sync.dma_start(
                        out=et_b[:, fi * ECAT:(fi + 1) * ECAT],
                        in_=eT_v[:, fi])
                et = [et_b[:, fi * ECAT:(fi + 1) * ECAT] for fi in range(FT)]

                # K^T[fo, col] over fi; N split 320+320
                for fot in range(FT):
                    wk_b = pa.tile([128, FT * 128], BF16, tag="wk", bufs=3,
                                   name=f"wk{fot}")
                    nc.sync.dma_start(
                        out=wk_b.rearrange("p (f o) -> p f o", f=FT),
                        in_=wkT_v[:, :, fot * 128:(fot + 1) * 128])
                    for nch in range(2):
                        n0 = nch * 320
                        pk = pap.tile([128, 320], F32, tag="pk", bufs=2,
                                      name=f"pk{fot}_{nch}")
                        for fi in range(FT):
                            nc.tensor.matmul(
                                pk[:], wk_b[:, fi * 128:(fi + 1) * 128],
                                et[fi][:, n0:n0 + 320],
                                start=(fi == 0), stop=(fi == FT - 1))
                        nc.scalar.copy(kt_sb[fot][:, n0:n0 + 320], pk[:])

                # V (natural layout [e, dv], bf16) over fi; tails of all 4
                # components go concurrently to disjoint column groups
                for fvc in range(3):
                    wv_b = pa.tile([128, FT * 512], BF16, tag="wv", bufs=2,
                                   name=f"wv{fvc}")
                    nc.sync.dma_start(
                        out=wv_b.rearrange("p (f o) -> p f o", f=FT),
                        in_=wvT_v[:, :, fvc * 512:(fvc + 1) * 512])
                    for c in range(NCOMP):
                        pv = pap.tile([128, 512], F32, tag="pv", bufs=2,
                                      name=f"pv{fvc}_{c}")
                        for fi in range(FT):
                            nc.tensor.matmul(
                                pv[:],
                                et[fi][:, c * EM:(c + 1) * EM],
                                wv_b[:, fi * 512:(fi + 1) * 512],
                                start=(fi == 0), stop=(fi == FT - 1))
                        nc.scalar.copy(
                            vm_sb[c][:, fvc * 512:(fvc + 1) * 512], pv[:])
                    pvt = pap.tile([128, 512], F32, tag="pv", bufs=2,
                                   name=f"pvt{fvc}")
                    for fi in range(FT):
                        for c in range(NCOMP):
                            nc.tensor.matmul(
                                pvt[c * 32:(c + 1) * 32, :],
                                et[fi][:, TB + c * 32:TB + (c + 1) * 32],
                                wv_b[:, fi * 512:(fi + 1) * 512],
                                start=(fi == 0), stop=(fi == FT - 1),
                                skip_group_check=True,
                                tile_position=(0, c * 32))
                    nc.scalar.copy(
                        vt_sb[:, fvc * 512:(fvc + 1) * 512], pvt[:])

            # ---------------- phase B: Q, scores, softmax, o ----------------
            with (
                tc.tile_pool(name="bc", bufs=1) as bc,
                tc.tile_pool(name="bcp", bufs=1, space="PSUM") as bcp,
            ):
                ot_sb = {}
                pend = None     # (hp, w_big, w_t) awaiting tail+AV

                def _tail_sum(hp, ex_t):
                    # Lones matmuls: tps[j,s] = sum_c ex_t[c*32+j, s]
                    rist = bc.tile([32, 2 * SL], F32, tag="rist", bufs=1,
                                   name=f"rist{hp}")
                    for sh in range(2):
                        tps = bcp.tile([128, 512], F32, tag="ps", bufs=3,
                                       name=f"tps{hp}_{sh}")
                        nc.tensor.matmul(tps[0:32, :], lones_sb[:],
                                         ex_t[:, sh * 512:(sh + 1) * 512],
                                         start=True, stop=True)
                        nc.vector.reciprocal_approx_fast(
                            out=rist[:, sh * 512:(sh + 1) * 512],
                            in_=tps[0:32, :])
                    ristb = bc.tile([32, 2 * SL], BF16, tag="ristb", bufs=1,
                                    name=f"ristb{hp}")
                    nc.vector.tensor_copy(out=ristb[:], in_=rist[:])
                    return ristb

                def _tail_mul_av(hp, w_big, ex_t, ristb):
                    # repl matmuls re-broadcast 1/sum across the 4 component
                    # partition groups (pad rows get 0); then AV
                    for sh in range(2):
                        rep = bcp.tile([128, 512], F32, tag="ps", bufs=3,
                                       name=f"rep{hp}_{sh}")
                        nc.tensor.matmul(
                            rep[:], repl_sb[:],
                            ristb[:, sh * 512:(sh + 1) * 512],
                            start=True, stop=True)
                        nc.vector.tensor_mul(
                            out=ex_t[:, sh * 512:(sh + 1) * 512],
                            in0=ex_t[:, sh * 512:(sh + 1) * 512], in1=rep[:])
                    w_t = ex_t  # normalized in place
                    for c in range(NCOMP if blevel >= 4 else 0):
                        po = bcp.tile([128, SL], F32, tag="po", bufs=3,
                                      name=f"po{hp}_{c}")
                        for hh in range(2):
                            h = hp * 2 + hh
                            nc.tensor.matmul(
                                po[hh * 64:(hh + 1) * 64, :],
                                vm_sb[c][:, h * 64:(h + 1) * 64],
                                w_big[:, c, hh * SL:(hh + 1) * SL],
                                start=True, stop=False,
                                skip_group_check=True)
                            nc.tensor.matmul(
                                po[hh * 64:(hh + 1) * 64, :],
                                vt_sb[c * 32:c * 32 + ET,
                                      h * 64:(h + 1) * 64],
                                w_t[c * 32:c * 32 + ET,
                                    hh * SL:(hh + 1) * SL],
                                start=False, stop=True,
                                skip_group_check=True,
                                tile_position=(c * 32, hh * 64))
                        ot = bc.tile([128, SL], BF16, tag="oT", bufs=48,
                                     name=f"ot{hp}_{c}")
                        nc.vector.tensor_copy(out=ot[:], in_=po[:])
                        ot_sb[(c, hp)] = ot

                for hp in range(HP if "B" in phases else 0):
                    if hp == 0:
                        wq_b = wq00
                    else:
                        wq_b = bc.tile([128, FT * 128], BF16, tag="wq",
                                       bufs=3, name=f"wq{hp}")
                        nc.sync.dma_start(
                            out=wq_b.rearrange("p (f o) -> p f o", f=FT),
                            in_=wqT_v[:, :, hp * 128:(hp + 1) * 128])

                    # Q^T chains, interleaved with the previous hp's
                    # deferred tail-softmax ops so the PE never waits
                    # on the scalar/vector engines
                    qt = []
                    rist_prev = None
                    for c in range(NCOMP):
                        pq = bcp.tile([128, SL], F32, tag="pq", bufs=2,
                                      name=f"pq{hp}_{c}")
                        for fi in range(FT):
                            nc.tensor.matmul(
                                pq[:], wq_b[:, fi * 128:(fi + 1) * 128],
                                xh[c][:, fi * SL:(fi + 1) * SL],
                                start=(fi == 0), stop=(fi == FT - 1))
                        q = bc.tile([128, SL], BF16, tag="qT", bufs=8,
                                    name=f"qt{hp}_{c}")
                        nc.scalar.copy(q[:], pq[:])
                        qt.append(q)
                        if c == 1 and pend is not None:
                            rist_prev = _tail_sum(pend[0], pend[2])
                    if pend is not None:
                        _tail_mul_av(pend[0], pend[1], pend[2], rist_prev)
                        pend = None
                    if blevel < 2:
                        continue

                    # scores + exp; main part: c along the free dim of
                    # one big tile, tail: c packed along partitions
                    ex_big = bc.tile([128, NCOMP, 2 * SL], BF16, tag="exb",
                                     bufs=2, name=f"exb{hp}")
                    for c in range(NCOMP):
                        for hh in range(2):
                            ps = bcp.tile([128, SL], F32, tag="ps", bufs=3,
                                          name=f"ps{hp}_{c}_{hh}")
                            nc.tensor.matmul(
                                ps[:],
                                kt_sb[hp][hh * 64:(hh + 1) * 64,
                                          c * EM:(c + 1) * EM],
                                qt[c][hh * 64:(hh + 1) * 64, :],
                                start=True, stop=True)
                            # heads packed side by side along the free dim
                            nc.scalar.activation(
                                ex_big[:, c, hh * 512:(hh + 1) * 512],
                                ps[:], AF.Exp)
                    ex_t = bc.tile([128, 2 * SL], BF16, tag="ext",
                                   bufs=2, name=f"ext{hp}")
                    for hh in range(2):
                        pst = bcp.tile([128, SL], F32, tag="ps", bufs=3,
                                       name=f"pst{hp}_{hh}")
                        for c in range(NCOMP):
                            nc.tensor.matmul(
                                pst[c * 32:(c + 1) * 32, :],
                                kt_sb[hp][hh * 64:(hh + 1) * 64,
                                          TB + c * 32:TB + (c + 1) * 32],
                                qt[c][hh * 64:(hh + 1) * 64, :],
                                start=True, stop=True,
                                skip_group_check=True,
                                tile_position=(hh * 64, c * 32))
                        nc.scalar.activation(
                            ex_t[:, hh * 512:(hh + 1) * 512], pst[:], AF.Exp)

                    # ---- cross-component softmax, main part (c on the
                    # free dim -> fused wide DVE ops) ----
                    padd = bc.tile([128, 2, 2 * SL], BF16, tag="padd", bufs=1,
                                   name=f"pa{hp}")
                    nc.vector.tensor_add(out=padd[:], in0=ex_big[:, 0:2, :],
                                         in1=ex_big[:, 2:4, :])
                    ssum = bc.tile([128, 2 * SL], F32, tag="ssum", bufs=1,
                                   name=f"sm{hp}")
                    nc.vector.tensor_add(out=ssum[:], in0=padd[:, 0, :],
                                         in1=padd[:, 1, :])
                    rinv = bc.tile([128, 2 * SL], F32, tag="rinv", bufs=1,
                                   name=f"ri{hp}")
                    nc.vector.reciprocal_approx_fast(out=rinv[:], in_=ssum[:])
                    rinvb = bc.tile([128, 2 * SL], BF16, tag="rinvb", bufs=1,
                                    name=f"rb{hp}")
                    nc.vector.tensor_copy(out=rinvb[:], in_=rinv[:])
                    # normalize in place: ex_big becomes w_big
                    nc.vector.tensor_mul(
                        out=ex_big[:], in0=ex_big[:],
                        in1=rinvb.unsqueeze(1).broadcast_to(
                            [128, NCOMP, 2 * SL]))
                    # ex_t is [128, 4*512] with hh along free; view the AV
                    # slices as [128, SL] via the hh packing
                    pend = (hp, ex_big, ex_t)

                if pend is not None:
                    rist_prev = _tail_sum(pend[0], pend[2])
                    _tail_mul_av(pend[0], pend[1], pend[2], rist_prev)
                    pend = None

                # -------- phase C: out-proj + bias + residual --------
                for fot in range(FT if "C" in phases else 0):
                    wo_b = bc.tile([128, FT * 128], BF16, tag="wo", bufs=3,
                                   name=f"wo{fot}")
                    nc.scalar.dma_start(
                        out=wo_b.rearrange("p (f o) -> p f o", f=FT),
                        in_=woT_v[:, :, fot * 128:(fot + 1) * 128])
                    for c in range(NCOMP):
                        po = bcp.tile([128, SL], F32, tag="po", bufs=3,
                                      name=f"pc{fot}_{c}")
                        for fi in range(FT):
                            nc.tensor.matmul(
                                po[:], wo_b[:, fi * 128:(fi + 1) * 128],
                                ot_sb[(c, fi)][:],
                                start=(fi == 0), stop=(fi == FT - 1))
                        ob = bc.tile([128, SL], F32, tag="outsb", bufs=3,
                                     name=f"ob{fot}_{c}")
                        nc.vector.scalar_tensor_tensor(
                            out=ob[:], in0=po[:],
                            scalar=bo_sb[:, fot:fot + 1],
                            in1=xh[c][:, fot * SL:(fot + 1) * SL],
                            op0=ALU.add, op1=ALU.add)
                        nc.sync.dma_start(
                            out=outT_v[c][:, fot, :], in_=ob[:])

        repeat = int(os.environ.get("K_REPEAT", "1"))
        for _rep in range(repeat):
            _phases()


_NC_CACHE = {}


def _get_nc():
    if "nc" not in _NC_CACHE:
        nc = bacc.Bacc("TRN2", target_bir_lowering=False)
        with tile.TileContext(nc) as tc:
            _emit(tc)
        nc.compile()
        _NC_CACHE["nc"] = nc
    return _NC_CACHE["nc"]


def kernel(hidden_states, encoder_hidden_states, temperature, Wq, Wk, Wv, Wo,
           bo, pad_length):
    # pad branch contributes zero to the output (zeros projected with no
    # bias give k_pad = v_pad = 0), so pad_length is irrelevant.
    hs = np.ascontiguousarray(np.asarray(hidden_states, dtype=np.float32))
    ehs = np.ascontiguousarray(
        np.asarray(encoder_hidden_states, dtype=np.float32))
    temp = float(np.asarray(temperature).reshape(-1)[0])
    Wq = np.asarray(Wq, dtype=np.float32)
    Wk = np.asarray(Wk, dtype=np.float32)
    Wv = np.asarray(Wv, dtype=np.float32)
    Wo = np.asarray(Wo, dtype=np.float32)
    bo_v = np.asarray(bo, dtype=np.float32).reshape(-1)

    wqT = np.ascontiguousarray((Wq / (temp + EPS)).T).astype(ml_dtypes.bfloat16)
    wkT = np.ascontiguousarray(Wk.T).astype(ml_dtypes.bfloat16)
    wvT = np.ascontiguousarray(Wv.T).astype(ml_dtypes.bfloat16)
    woT = np.ascontiguousarray(Wo.T).astype(ml_dtypes.bfloat16)
    eT_all = np.zeros((D, ECAT), dtype=np.float32)
    for c in range(NCOMP):
        eT_all[:, c * EM:(c + 1) * EM] = ehs[c].T[:, :EM]
        eT_all[:, TB + c * 32:TB + c * 32 + ET] = ehs[c].T[:, EM:E]
    eT_all = eT_all.astype(ml_dtypes.bfloat16)
    bo_t = np.ascontiguousarray(bo_v.reshape(FT, 128).T)

    # lones sums the 4 component groups: tps[j] = sum_c ex[c*32+j];
    # rows j>=26 get a benign positive value (ex[0]) so 1/x stays finite
    lones_h = np.zeros((128, 32), dtype=np.float32)
    for c in range(NCOMP):
        for j in range(ET):
            lones_h[c * 32 + j, j] = 1.0
    for j in range(ET, 32):
        lones_h[0, j] = 1.0
    # repl re-broadcasts: rep[c*32+j] = rinv[j] for j<26, 0 on pad rows
    repl_h = np.zeros((32, 128), dtype=np.float32)
    for c in range(NCOMP):
        for j in range(ET):
            repl_h[j, c * 32 + j] = 1.0

    nc = _get_nc()
    in_maps = []
    for i in range(NCORES):
        xT_i = np.ascontiguousarray(
            hs[:, i * SL:(i + 1) * SL, :].transpose(0, 2, 1)).astype(
                ml_dtypes.bfloat16)
        in_maps.append({
            "xTb": xT_i, "eT": eT_all, "wqT": wqT, "wkT": wkT,
            "wvT": wvT, "woT": woT, "bo": bo_t,
            "lones": lones_h.astype(ml_dtypes.bfloat16),
            "repl": repl_h.astype(ml_dtypes.bfloat16),
        })

    res = run_bass_kernel_spmd(nc, in_maps, core_ids=list(range(NCORES)))

    out = np.empty((NCOMP, S, D), dtype=np.float32)
    for i in range(NCORES):
        out[:, i * SL:(i + 1) * SL, :] = res.results[i]["outT"].transpose(
            0, 2, 1)
    return out


# revision 34
# speedup vs baseline: 1.0234x; 1.0234x over previous
"""Trainium2 Bass kernel for DecomposingAttnProcessor (pad variant).

Math (pad branch contributes exactly zero since pad tokens are zeros
projected with no bias -> k_pad = v_pad = 0):
    q = hs @ Wq.T / (temp + eps)   (scale folded into Wq on host)
    k = ehs @ Wk.T ; v = ehs @ Wv.T
    scores[c,h,s,e] = q . k        (per head, dh=64)
    w = softmax over the 4 components c (dim 0)
    o = w @ v ; out = o @ Wo.T + bo + hs

Sharding: 8 cores, split S=4096 into 512-row blocks; all 4 components of
a block stay on one core (softmax couples them). K/V computed redundantly
per core (encoder seq is only 154).

All matmuls run at N=512 (one full PSUM bank) to amortize the per-matmul
LDWEIGHTS/dispatch tax; the whole 512-row s-block is processed in one
pass (no s-halves).

Encoder layout (host-packed, 640 columns):
    cols [c*128,(c+1)*128) = component c, e in [0,128)   ("main")
    cols [512+c*32, 512+c*32+32) = component c, e in [128,154) zero-padded
    to 32 ("tail") so every matmul operand is 32-aligned on partitions.
Tail matmuls for the 4 components target disjoint PE sub-arrays
(tile_position) and run concurrently instead of serially at M=26.

The tail softmax sums over components and re-broadcasts 1/sum with two
tiny constant matmuls (lones / repl) because DVE tensor_tensor requires
both SBUF inputs to share a start partition; PSUM operands are exempt.
"""

import numpy as np
import ml_dtypes

import concourse.bass as bass
import concourse.mybir as mybir
import concourse.tile as tile
from concourse import bacc
from concourse.bass_utils import run_bass_kernel_spmd

F32 = mybir.dt.float32
BF16 = mybir.dt.bfloat16
AF = mybir.ActivationFunctionType
ALU = mybir.AluOpType

NCOMP = 4
HEADS = 24
DH = 64
D = 1536
S = 4096
E = 154
EM = 128                  # main e-rows per component
ET = E - EM               # 26 tail e-rows per component
EPS = 1e-8
NCORES = 8
SL = S // NCORES          # 512 s-rows per core (per component)
FT = D // 128             # 12 feature tiles of 128
HP = HEADS // 2           # 12 head-pairs (2 heads = 128 feature rows)
ECAT = 640                # 4*128 main + 4*32 padded tail columns
TB = 4 * EM               # 512: tail block column base


def _emit(tc):
    import os
    phases = os.environ.get("K_PHASES", "ABC")
    blevel = int(os.environ.get("K_BLEVEL", "4"))
    nc = tc.nc

    xTb = nc.declare_dram_parameter("xTb", [NCOMP, D, SL], BF16, isOutput=False)
    eT = nc.declare_dram_parameter("eT", [D, ECAT], BF16, isOutput=False)
    wqT = nc.declare_dram_parameter("wqT", [D, D], BF16, isOutput=False)
    wkT = nc.declare_dram_parameter("wkT", [D, D], BF16, isOutput=False)
    wvT = nc.declare_dram_parameter("wvT", [D, D], BF16, isOutput=False)
    woT = nc.declare_dram_parameter("woT", [D, D], BF16, isOutput=False)
    bo = nc.declare_dram_parameter("bo", [128, FT], F32, isOutput=False)
    lones = nc.declare_dram_parameter("lones", [128, 32], BF16, isOutput=False)
    repl = nc.declare_dram_parameter("repl", [32, 128], BF16, isOutput=False)
    outT = nc.declare_dram_parameter("outT", [NCOMP, D, SL], F32, isOutput=True)

    xTb_v = [xTb[c].rearrange("(f p) s -> p f s", p=128) for c in range(NCOMP)]
    eT_v = eT.rearrange("(f p) e -> p f e", p=128)
    wqT_v = wqT.rearrange("(f p) o -> p f o", p=128)
    wkT_v = wkT.rearrange("(f p) o -> p f o", p=128)
    wvT_v = wvT.rearrange("(f p) o -> p f o", p=128)
    woT_v = woT.rearrange("(f p) o -> p f o", p=128)
    outT_v = [outT[c].rearrange("(f p) s -> p f s", p=128) for c in range(NCOMP)]

    with tc.tile_pool(name="persist", bufs=1) as pp:
        # ---------------- persistent tiles ----------------
        kt_sb = [pp.tile([128, ECAT], BF16, tag="kT", bufs=FT, name=f"kt{t}")
                 for t in range(FT)]
        vm_sb = [pp.tile([128, D], BF16, tag="vm", bufs=NCOMP, name=f"vm{c}")
                 for c in range(NCOMP)]
        vt_sb = pp.tile([128, D], BF16, tag="vt", bufs=1, name="vt")
        bo_sb = pp.tile([128, FT], F32, tag="bo", bufs=1, name="bo_sb")
        nc.sync.dma_start(out=bo_sb[:], in_=bo[:])
        lones_sb = pp.tile([128, 32], BF16, tag="lones", bufs=1, name="lones_sb")
        nc.sync.dma_start(out=lones_sb[:], in_=lones[:])
        repl_sb = pp.tile([32, 128], BF16, tag="repl", bufs=1, name="repl_sb")
        nc.sync.dma_start(out=repl_sb[:], in_=repl[:])

        # x panels (Q rhs + residual source) and the first wq block live in
        # the persistent pool; their DMAs are issued interleaved into phase
        # A so the bulk transfers never sit ahead of the latency-critical
        # eT/wk loads in the sync queue
        xh = [pp.tile([128, FT * SL], BF16, tag="xh", bufs=NCOMP,
                      name=f"xh_{c}") for c in range(NCOMP)]
        wq00 = pp.tile([128, FT * 128], BF16, tag="wq00", bufs=1, name="wq00")

        def _load_xh(c):
            nc.sync.dma_start(
                out=xh[c].rearrange("p (f s) -> p f s", f=FT), in_=xTb_v[c])

        def _load_wq00():
            nc.sync.dma_start(
                out=wq00.rearrange("p (f o) -> p f o", f=FT),
                in_=wqT_v[:, :, 0:128])

        def _phases():
            # ---------------- phase A: K^T and V ----------------
            if "A" in phases:
              with (
                tc.tile_pool(name="pha", bufs=1) as pa,
                tc.tile_pool(name="pha_psum", bufs=1, space="PSUM") as pap,
              ):
                et_b = pa.tile([128, FT * ECAT], BF16, tag="eT", bufs=1,
                               name="et_b")
                # per-fi DMAs so the first K^T chain starts early
                for fi in range(FT):
                    nc.sync.dma_start(
                        out=et_b[:, fi * ECAT:(fi + 1) * ECAT],
                        in_=eT_v[:, fi])
                et = [et_b[:, fi * ECAT:(fi + 1) * ECAT] for fi in range(FT)]

                # K^T[fo, col] over fi; N split 320+320
                for fot in range(FT):
                    wk_b = pa.tile([128, FT * 128], BF16, tag="wk", bufs=3,
                                   name=f"wk{fot}")
                    nc.sync.dma_start(
                        out=wk_b.rearrange("p (f o) -> p f o", f=FT),
                        in_=wkT_v[:, :, fot * 128:(fot + 1) * 128])
                    for nch in range(2):
                        n0 = nch * 320
                        pk = pap.tile([128, 320], F32, tag="pk", bufs=2,
                                      name=f"pk{fot}_{nch}")
                        for fi in range(FT):
                            nc.tensor.matmul(
                                pk[:], wk_b[:, fi * 128:(fi + 1) * 128],
                                et[fi][:, n0:n0 + 320],
                                start=(fi == 0), stop=(fi == FT - 1))
                        nc.scalar.copy(kt_sb[fot][:, n0:n0 + 320], pk[:])
                    if fot % 3 == 2:
                        _load_xh(fot // 3)

                # V (natural layout [e, dv], bf16) over fi; tails of all 4
                # components go concurrently to disjoint column groups
                for fvc in range(3):
                    wv_b = pa.tile([128, FT * 512], BF16, tag="wv", bufs=2,
                                   name=f"wv{fvc}")
                    nc.sync.dma_start(
                        out=wv_b.rearrange("p (f o) -> p f o", f=FT),
                        in_=wvT_v[:, :, fvc * 512:(fvc + 1) * 512])
                    for c in range(NCOMP):
                        pv = pap.tile([128, 512], F32, tag="pv", bufs=2,
                                      name=f"pv{fvc}_{c}")
                        for fi in range(FT):
                            nc.tensor.matmul(
                                pv[:],
                                et[fi][:, c * EM:(c + 1) * EM],
                                wv_b[:, fi * 512:(fi + 1) * 512],
                                start=(fi == 0), stop=(fi == FT - 1))
                        nc.scalar.copy(
                            vm_sb[c][:, fvc * 512:(fvc + 1) * 512], pv[:])
                    pvt = pap.tile([128, 512], F32, tag="pv", bufs=2,
                                   name=f"pvt{fvc}")
                    for fi in range(FT):
                        for c in range(NCOMP):
                            nc.tensor.matmul(
                                pvt[c * 32:(c + 1) * 32, :],
                                et[fi][:, TB + c * 32:TB + (c + 1) * 32],
                                wv_b[:, fi * 512:(fi + 1) * 512],
                                start=(fi == 0), stop=(fi == FT - 1),
                                skip_group_check=True,
                                tile_position=(0, c * 32))
                    nc.scalar.copy(
                        vt_sb[:, fvc * 512:(fvc + 1) * 512], pvt[:])
                    if fvc == 0:
                        _load_wq00()
            if "A" not in phases:
                for c in range(NCOMP):
                    _load_xh(c)
                _load_wq00()

            # ---------------- phase B: Q, scores, softmax, o ----------------
            with (
                tc.tile_pool(name="bc", bufs=1) as bc,
                tc.tile_pool(name="bcp", bufs=1, space="PSUM") as bcp,
            ):
                ot_sb = {}
                pend = None     # (hp, w_big, w_t) awaiting tail+AV

                # pre-issue the first wo loads so phase C starts without a
                # DMA wait (ring slots recycle for fot >= 3)
                wo_pre = []
                for fot in range(3 if "C" in phases else 0):
                    wo_b = bc.tile([128, FT * 128], BF16, tag="wo", bufs=3,
                                   name=f"wo{fot}")
                    nc.sync.dma_start(
                        out=wo_b.rearrange("p (f o) -> p f o", f=FT),
                        in_=woT_v[:, :, fot * 128:(fot + 1) * 128])
                    wo_pre.append(wo_b)

                def _tail_sum(hp, ex_t):
                    # Lones matmuls: tps[j,s] = sum_c ex_t[c*32+j, s]
                    rist = bc.tile([32, 2 * SL], F32, tag="rist", bufs=1,
                                   name=f"rist{hp}")
                    for sh in range(2):
                        tps = bcp.tile([128, 512], F32, tag="ps", bufs=3,
                                       name=f"tps{hp}_{sh}")
                        nc.tensor.matmul(tps[0:32, :], lones_sb[:],
                                         ex_t[:, sh * 512:(sh + 1) * 512],
                                         start=True, stop=True)
                        nc.vector.reciprocal_approx_fast(
                            out=rist[:, sh * 512:(sh + 1) * 512],
                            in_=tps[0:32, :])
                    ristb = bc.tile([32, 2 * SL], BF16, tag="ristb", bufs=1,
                                    name=f"ristb{hp}")
                    nc.vector.tensor_copy(out=ristb[:], in_=rist[:])
                    return ristb

                def _tail_mul_av(hp, w_big, ex_t, ristb):
                    # repl matmuls re-broadcast 1/sum across the 4 component
                    # partition groups (pad rows get 0); then AV
                    for sh in range(2):
                        rep = bcp.tile([128, 512], F32, tag="ps", bufs=3,
                                       name=f"rep{hp}_{sh}")
                        nc.tensor.matmul(
                            rep[:], repl_sb[:],
                            ristb[:, sh * 512:(sh + 1) * 512],
                            start=True, stop=True)
                        nc.vector.tensor_mul(
                            out=ex_t[:, sh * 512:(sh + 1) * 512],
                            in0=ex_t[:, sh * 512:(sh + 1) * 512], in1=rep[:])
                    w_t = ex_t  # normalized in place
                    for c in range(NCOMP if blevel >= 4 else 0):
                        po = bcp.tile([128, SL], F32, tag="po", bufs=3,
                                      name=f"po{hp}_{c}")
                        for hh in range(2):
                            h = hp * 2 + hh
                            nc.tensor.matmul(
                                po[hh * 64:(hh + 1) * 64, :],
                                vm_sb[c][:, h * 64:(h + 1) * 64],
                                w_big[:, c, hh * SL:(hh + 1) * SL],
                                start=True, stop=False,
                                skip_group_check=True)
                            nc.tensor.matmul(
                                po[hh * 64:(hh + 1) * 64, :],
                                vt_sb[c * 32:c * 32 + ET,
                                      h * 64:(h + 1) * 64],
                                w_t[c * 32:c * 32 + ET,
                                    hh * SL:(hh + 1) * SL],
                                start=False, stop=True,
                                skip_group_check=True,
                                tile_position=(c * 32, hh * 64))
                        ot = bc.tile([128, SL], BF16, tag="oT", bufs=48,
                                     name=f"ot{hp}_{c}")
                        nc.vector.tensor_copy(out=ot[:], in_=po[:])
                        ot_sb[(c, hp)] = ot

                for hp in range(HP if "B" in phases else 0):
                    if hp == 0:
                        wq_b = wq00
                    else:
                        wq_b = bc.tile([128, FT * 128], BF16, tag="wq",
                                       bufs=3, name=f"wq{hp}")
                        nc.sync.dma_start(
                            out=wq_b.rearrange("p (f o) -> p f o", f=FT),
                            in_=wqT_v[:, :, hp * 128:(hp + 1) * 128])

                    # Q^T chains, interleaved with the previous hp's
                    # deferred tail-softmax ops so the PE never waits
                    # on the scalar/vector engines
                    qt = []
                    rist_prev = None
                    for c in range(NCOMP):
                        pq = bcp.tile([128, SL], F32, tag="pq", bufs=2,
                                      name=f"pq{hp}_{c}")
                        for fi in range(FT):
                            nc.tensor.matmul(
                                pq[:], wq_b[:, fi * 128:(fi + 1) * 128],
                                xh[c][:, fi * SL:(fi + 1) * SL],
                                start=(fi == 0), stop=(fi == FT - 1))
                        q = bc.tile([128, SL], BF16, tag="qT", bufs=8,
                                    name=f"qt{hp}_{c}")
                        nc.scalar.copy(q[:], pq[:])
                        qt.append(q)
                        if c == 1 and pend is not None:
                            rist_prev = _tail_sum(pend[0], pend[2])
                    if pend is not None:
                        _tail_mul_av(pend[0], pend[1], pend[2], rist_prev)
                        pend = None
                    if blevel < 2:
                        continue

                    # scores + exp; main part: c along the free dim of
                    # one big tile, tail: c packed along partitions
                    ex_big = bc.tile([128, NCOMP, 2 * SL], BF16, tag="exb",
                                     bufs=2, name=f"exb{hp}")
                    for c in range(NCOMP):
                        for hh in range(2):
                            ps = bcp.tile([128, SL], F32, tag="ps", bufs=3,
                                          name=f"ps{hp}_{c}_{hh}")
                            nc.tensor.matmul(
                                ps[:],
                                kt_sb[hp][hh * 64:(hh + 1) * 64,
                                          c * EM:(c + 1) * EM],
                                qt[c][hh * 64:(hh + 1) * 64, :],
                                start=True, stop=True)
                            # heads packed side by side along the free dim
                            nc.scalar.activation(
                                ex_big[:, c, hh * 512:(hh + 1) * 512],
                                ps[:], AF.Exp)
                    ex_t = bc.tile([128, 2 * SL], BF16, tag="ext",
                                   bufs=2, name=f"ext{hp}")
                    for hh in range(2):
                        pst = bcp.tile([128, SL], F32, tag="ps", bufs=3,
                                       name=f"pst{hp}_{hh}")
                        for c in range(NCOMP):
                            nc.tensor.matmul(
                                pst[c * 32:(c + 1) * 32, :],
                                kt_sb[hp][hh * 64:(hh + 1) * 64,
                                          TB + c * 32:TB + (c + 1) * 32],
                                qt[c][hh * 64:(hh + 1) * 64, :],
                                start=True, stop=True,
                                skip_group_check=True,
                                tile_position=(hh * 64, c * 32))
                        nc.scalar.activation(
                            ex_t[:, hh * 512:(hh + 1) * 512], pst[:], AF.Exp)

                    # ---- cross-component softmax, main part (c on the
                    # free dim -> fused wide DVE ops) ----
                    padd = bc.tile([128, 2, 2 * SL], BF16, tag="padd", bufs=1,
                                   name=f"pa{hp}")
                    nc.vector.tensor_add(out=padd[:], in0=ex_big[:, 0:2, :],
                                         in1=ex_big[:, 2:4, :])
                    ssum = bc.tile([128, 2 * SL], F32, tag="ssum", bufs=1,
                                   name=f"sm{hp}")
                    nc.vector.tensor_add(out=ssum[:], in0=padd[:, 0, :],
                                         in1=padd[:, 1, :])
                    rinv = bc.tile([128, 2 * SL], F32, tag="rinv", bufs=1,
                                   name=f"ri{hp}")
                    nc.vector.reciprocal_approx_fast(out=rinv[:], in_=ssum[:])
                    rinvb = bc.tile([128, 2 * SL], BF16, tag="rinvb", bufs=1,
                                    name=f"rb{hp}")
                    nc.vector.tensor_copy(out=rinvb[:], in_=rinv[:])
                    # normalize in place: ex_big becomes w_big
                    nc.vector.tensor_mul(
                        out=ex_big[:], in0=ex_big[:],
                        in1=rinvb.unsqueeze(1).broadcast_to(
                            [128, NCOMP, 2 * SL]))
                    # ex_t is [128, 4*512] with hh along free; view the AV
                    # slices as [128, SL] via the hh packing
                    pend = (hp, ex_big, ex_t)

                if pend is not None:
                    rist_prev = _tail_sum(pend[0], pend[2])
                    _tail_mul_av(pend[0], pend[1], pend[2], rist_prev)
                    pend = None

                # -------- phase C: out-proj + bias + residual --------
                for fot in range(FT if "C" in phases else 0):
                    if fot < 3:
                        wo_b = wo_pre[fot]
                    else:
                        wo_b = bc.tile([128, FT * 128], BF16, tag="wo",
                                       bufs=3, name=f"wo{fot}")
                        nc.sync.dma_start(
                            out=wo_b.rearrange("p (f o) -> p f o", f=FT),
                            in_=woT_v[:, :, fot * 128:(fot + 1) * 128])
                    for c in range(NCOMP):
                        po = bcp.tile([128, SL], F32, tag="po", bufs=3,
                                      name=f"pc{fot}_{c}")
                        for fi in range(FT):
                            nc.tensor.matmul(
                                po[:], wo_b[:, fi * 128:(fi + 1) * 128],
                                ot_sb[(c, fi)][:],
                                start=(fi == 0), stop=(fi == FT - 1))
                        ob = bc.tile([128, SL], F32, tag="outsb", bufs=3,
                                     name=f"ob{fot}_{c}")
                        nc.vector.scalar_tensor_tensor(
                            out=ob[:], in0=po[:],
                            scalar=bo_sb[:, fot:fot + 1],
                            in1=xh[c][:, fot * SL:(fot + 1) * SL],
                            op0=ALU.add, op1=ALU.add)
                        nc.sync.dma_start(
                            out=outT_v[c][:, fot, :], in_=ob[:])

        repeat = int(os.environ.get("K_REPEAT", "1"))
        for _rep in range(repeat):
            _phases()


_NC_CACHE = {}


def _get_nc():
    if "nc" not in _NC_CACHE:
        nc = bacc.Bacc("TRN2", target_bir_lowering=False)
        with tile.TileContext(nc) as tc:
            _emit(tc)
        nc.compile()
        _NC_CACHE["nc"] = nc
    return _NC_CACHE["nc"]


def kernel(hidden_states, encoder_hidden_states, temperature, Wq, Wk, Wv, Wo,
           bo, pad_length):
    # pad branch contributes zero to the output (zeros projected with no
    # bias give k_pad = v_pad = 0), so pad_length is irrelevant.
    hs = np.ascontiguousarray(np.asarray(hidden_states, dtype=np.float32))
    ehs = np.ascontiguousarray(
        np.asarray(encoder_hidden_states, dtype=np.float32))
    temp = float(np.asarray(temperature).reshape(-1)[0])
    Wq = np.asarray(Wq, dtype=np.float32)
    Wk = np.asarray(Wk, dtype=np.float32)
    Wv = np.asarray(Wv, dtype=np.float32)
    Wo = np.asarray(Wo, dtype=np.float32)
    bo_v = np.asarray(bo, dtype=np.float32).reshape(-1)

    wqT = np.ascontiguousarray((Wq / (temp + EPS)).T).astype(ml_dtypes.bfloat16)
    wkT = np.ascontiguousarray(Wk.T).astype(ml_dtypes.bfloat16)
    wvT = np.ascontiguousarray(Wv.T).astype(ml_dtypes.bfloat16)
    woT = np.ascontiguousarray(Wo.T).astype(ml_dtypes.bfloat16)
    eT_all = np.zeros((D, ECAT), dtype=np.float32)
    for c in range(NCOMP):
        eT_all[:, c * EM:(c + 1) * EM] = ehs[c].T[:, :EM]
        eT_all[:, TB + c * 32:TB + c * 32 + ET] = ehs[c].T[:, EM:E]
    eT_all = eT_all.astype(ml_dtypes.bfloat16)
    bo_t = np.ascontiguousarray(bo_v.reshape(FT, 128).T)

    # lones sums the 4 component groups: tps[j] = sum_c ex[c*32+j];
    # rows j>=26 get a benign positive value (ex[0]) so 1/x stays finite
    lones_h = np.zeros((128, 32), dtype=np.float32)
    for c in range(NCOMP):
        for j in range(ET):
            lones_h[c * 32 + j, j] = 1.0
    for j in range(ET, 32):
        lones_h[0, j] = 1.0
    # repl re-broadcasts: rep[c*32+j] = rinv[j] for j<26, 0 on pad rows
    repl_h = np.zeros((32, 128), dtype=np.float32)
    for c in range(NCOMP):
        for j in range(ET):
            repl_h[j, c * 32 + j] = 1.0

    nc = _get_nc()
    in_maps = []
    for i in range(NCORES):
        xT_i = np.ascontiguousarray(
            hs[:, i * SL:(i + 1) * SL, :].transpose(0, 2, 1)).astype(
                ml_dtypes.bfloat16)
        in_maps.append({
            "xTb": xT_i, "eT": eT_all, "wqT": wqT, "wkT": wkT,
            "wvT": wvT, "woT": woT, "bo": bo_t,
            "lones": lones_h.astype(ml_dtypes.bfloat16),
            "repl": repl_h.astype(ml_dtypes.bfloat16),
        })

    res = run_bass_kernel_spmd(nc, in_maps, core_ids=list(range(NCORES)))

    out = np.empty((NCOMP, S, D), dtype=np.float32)
    for i in range(NCORES):
        out[:, i * SL:(i + 1) * SL, :] = res.results[i]["outT"].transpose(
            0, 2, 1)
    return out
